# revision 7
# baseline (speedup 1.0000x reference)
"""Trainium2 Bass kernel for causal self-attention (muP scaling).

Full-input contract: kernel(**inputs) takes the complete tensors and returns
the complete [B, T, C] output. Internally the work is split over 8 NeuronCores
as (batch b = core//2) x (head-group g = core%2, 8 heads each):

  - each core computes q,k,v for its batch restricted to its 8 heads,
    runs causal attention for those heads, and multiplies by the matching
    512-row slice of w_proj, producing a partial [T, C] output.
  - the host sums the two partials per batch and adds b_proj. No on-device
    collectives are needed.

Layout trick: the host passes x[b].T (i.e. [C, T]) so that
  - qT,kT ([dim, t]) come from matmuls with the weight slice as the
    stationary operand and xT as the moving operand,
  - v ([t, dim]) comes from matmuls with xT tiles as the stationary operand,
so no on-chip transposes are needed anywhere.

Attention per head: scoresT[tk, tq] = kT_tile.T @ qT (contraction over
head_dim=64, fp32r), exp on ScalarE straight out of PSUM (muP scale 1/64
folded into the activation scale; no max-subtraction - logits are ~N(0, .13)
so exp never overflows), causal masking with a 0/1 upper-triangular multiply
on diagonal-crossing tiles only (off-diagonal invalid tiles are never
computed). attT-out[d, tq] accumulates v_aug.T @ expT where v_aug carries an
appended ones column, so row 64 of the accumulator is the softmax denominator
for free. Normalization: reciprocal of that row, broadcast across the 64
head-dim partitions with a K=1 PE matmul, then one fused multiply while
copying PSUM->SBUF. The normalized attention output lands directly in
[c, t] layout, which is exactly the stationary-operand layout the final
projection wants.
"""

import sys

if "/opt/trn_rl_repo" not in sys.path:
    sys.path.insert(0, "/opt/trn_rl_repo")

import numpy as np

import concourse.bass as bass
import concourse.mybir as mybir
import concourse.tile as tile
from concourse import bacc
from concourse.bass_utils import run_bass_kernel_spmd
from concourse.masks import make_upper_triangular

# Problem shape (hardcoded per contract).
B, T, C, H = 4, 2048, 1024, 16
HD = C // H            # 64
N_CORES = 8
HG = H // 2            # 8 heads per core
GC = HG * HD           # 512 columns of q/k/v per core
P = 128                # SBUF partitions
CT = C // P            # 8 contraction tiles over C
TT = T // P            # 16 time tiles of 128
QB = 4                 # tq blocks
QW = T // QB           # 512 wide
KT = T // P            # 16 tk tiles

F32 = mybir.dt.float32
F32R = mybir.dt.float32r
BF16 = mybir.dt.bfloat16

_COMPILED = None


def _r(ap):
    """Reinterpret an fp32 AP as float32r for full-rate PE matmuls."""
    return ap.bitcast(F32R)


def _build_nc():
    nc = bacc.Bacc("TRN2", target_bir_lowering=False, debug=False,
                   num_devices=N_CORES)

    xT = nc.dram_tensor("xT", [C, T], F32, kind="ExternalInput").ap()
    w_qk = nc.dram_tensor("w_qk", [C, 2 * GC], F32, kind="ExternalInput").ap()
    w_v = nc.dram_tensor("w_v", [C, GC], F32, kind="ExternalInput").ap()
    b_qk = nc.dram_tensor("b_qk", [2 * GC], F32, kind="ExternalInput").ap()
    b_v = nc.dram_tensor("b_v", [GC], F32, kind="ExternalInput").ap()
    w_pr = nc.dram_tensor("w_pr", [GC, C], F32, kind="ExternalInput").ap()
    y = nc.dram_tensor("y", [T, C], F32, kind="ExternalOutput").ap()

    with tile.TileContext(nc) as tc:
        _emit(nc, tc, xT, w_qk, w_v, b_qk, b_v, w_pr, y)
    nc.finalize()
    return nc


def _emit(nc, tc, xT, w_qk, w_v, b_qk, b_v, w_pr, y):
    from contextlib import ExitStack

    ctx = ExitStack()
    with ctx:
        persist = ctx.enter_context(tc.tile_pool(name="persist", bufs=1))

        # ---- constants -------------------------------------------------
        tri = persist.tile([P, P], BF16, tag="tri")     # 0/1, 1 iff j >= i
        make_upper_triangular(nc, tri[:, :], val=1.0, diag=True)

        bqk_sb = persist.tile([P, CT], F32, tag="bqk")  # [128, 8] col jt
        nc.sync.dma_start(
            out=bqk_sb[:, :],
            in_=bass.AP(tensor=b_qk.tensor, offset=0, ap=[[1, P], [P, CT]]),
        )
        bv_sb = persist.tile([P, GC], F32, tag="bv")
        nc.gpsimd.dma_start(
            out=bv_sb[:, :],
            in_=bass.AP(tensor=b_v.tensor, offset=0, ap=[[0, P], [1, GC]]),
        )

        # ---- persistent activation buffers ----------------------------
        qkT = [persist.tile([P, T], F32R, name=f"qkT{j}", tag=f"qkT{j}") for j in range(CT)]
        v_sb = [persist.tile([P, HG, HD + 1], BF16, name=f"v{t}", tag=f"v{t}")
                for t in range(TT)]

        # ================= phase 1: qkv projections ====================
        with tc.tile_pool(name="xT", bufs=1) as xp:
            xts = []
            for ct in range(CT):
                xt = xp.tile([P, T], F32R, name=f"xT{ct}", tag=f"xT{ct}")
                nc.sync.dma_start(out=xt[:, :], in_=xT[ct * P:(ct + 1) * P, :].bitcast(F32R))
                xts.append(xt)

            with tc.tile_pool(name="wqk", bufs=1) as wp, \
                 tc.tile_pool(name="ps1", bufs=4, space="PSUM") as ps1:
                wts = []
                for ct in range(CT):
                    wt = wp.tile([P, 2 * GC], F32R, name=f"wqk{ct}", tag=f"wqk{ct}")
                    nc.sync.dma_start(out=wt[:, :],
                                      in_=w_qk[ct * P:(ct + 1) * P, :].bitcast(F32R))
                    wts.append(wt)
                for jt in range(CT):
                    for tb in range(QB):
                        ps = ps1.tile([P, QW], F32)
                        for ct in range(CT):
                            nc.tensor.matmul(
                                ps[:, :],
                                wts[ct][:, jt * P:(jt + 1) * P],
                                xts[ct][:, tb * QW:(tb + 1) * QW],
                                start=(ct == 0), stop=(ct == CT - 1),
                            )
                        nc.vector.tensor_scalar_add(
                            out=qkT[jt][:, tb * QW:(tb + 1) * QW],
                            in0=ps[:, :],
                            scalar1=bqk_sb[:, jt:jt + 1],
                        )

            with tc.tile_pool(name="wv", bufs=1) as wvp, \
                 tc.tile_pool(name="ps1v", bufs=4, space="PSUM") as ps1v:
                wvts = []
                for ct in range(CT):
                    wvt = wvp.tile([P, GC], F32R, name=f"wv{ct}", tag=f"wv{ct}")
                    nc.sync.dma_start(out=wvt[:, :],
                                      in_=w_v[ct * P:(ct + 1) * P, :].bitcast(F32R))
                    wvts.append(wvt)
                for tt in range(TT):
                    ps = ps1v.tile([P, GC], F32)
                    for ct in range(CT):
                        nc.tensor.matmul(
                            ps[:, :],
                            xts[ct][:, tt * P:(tt + 1) * P],
                            wvts[ct][:, :],
                            start=(ct == 0), stop=(ct == CT - 1),
                        )
                    nc.vector.tensor_add(
                        out=v_sb[tt][:, :, 0:HD],
                        in0=ps[:, :].rearrange("p (h e) -> p h e", e=HD),
                        in1=bv_sb[:, :].rearrange("p (h e) -> p h e", e=HD),
                    )
                    nc.vector.memset(v_sb[tt][:, :, HD:HD + 1], 1.0)

        # ================= phase 2: attention ==========================
        # Opened after the xT pool closes so its SBUF space is reused.
        ph23 = ctx.enter_context(tc.tile_pool(name="ph23", bufs=1))
        att = [ph23.tile([P, T], F32R, name=f"att{j}", tag=f"att{j}") for j in range(CT // 2)]
        wpr = [ph23.tile([P, C], F32R, name=f"wpr{j}", tag=f"wpr{j}") for j in range(CT // 2)]
        for ct in range(CT // 2):
            nc.sync.dma_start(out=wpr[ct][:, :], in_=w_pr[ct * P:(ct + 1) * P, :].bitcast(F32R))

        with tc.tile_pool(name="expp", bufs=24) as expp, \
             tc.tile_pool(name="nrm", bufs=4) as nrm, \
             tc.tile_pool(name="ps_s", bufs=4, space="PSUM") as pss, \
             tc.tile_pool(name="ps_o", bufs=2, space="PSUM") as pso:
            for h in range(HG):
                jt, half = h // 2, h % 2
                r0 = half * HD                       # partition base 0 / 64
                qT_t = qkT[jt]
                kT_t = qkT[CT // 2 + jt]
                for qb in range(QB):
                    q0 = qb * QW
                    # (tk tile index, column offset inside the block)
                    tiles = [(kt, 0, False) for kt in range(4 * qb)]
                    tiles += [(4 * qb + a, P * a, True) for a in range(4)]

                    acc = pso.tile([P, QW], F32)
                    exps = []
                    for kt, off, crossing in tiles:
                        n = QW - off
                        ps = pss.tile([P, QW], F32, tag="scores")
                        nc.tensor.matmul(
                            ps[:, 0:n],
                            kT_t[r0:r0 + HD, kt * P:(kt + 1) * P],
                            qT_t[r0:r0 + HD, q0 + off:q0 + QW],
                            start=True, stop=True,
                        )
                        ex = expp.tile([P, QW], BF16, tag="exp")
                        nc.scalar.activation(
                            out=ex[:, 0:n], in_=ps[:, 0:n],
                            func=mybir.ActivationFunctionType.Exp,
                            scale=1.0 / HD,
                        )
                        if crossing:
                            # diagonal-crossing tile: first 128 cols hold the
                            # triangle; zero the not-yet-valid upper part
                            nc.vector.tensor_mul(
                                out=ex[:, 0:P], in0=ex[:, 0:P], in1=tri[:, :])
                        exps.append((kt, off, n, ex))

                    for i, (kt, off, n, ex) in enumerate(exps):
                        nc.tensor.matmul(
                            acc[0:HD + 1, off:QW],
                            v_sb[kt][:, h, :],
                            ex[:, 0:n],
                            start=(i == 0), stop=(i == len(exps) - 1),
                            skip_group_check=True,
                        )

                    # normalize: rows 0:64 divided by the ones-row sum (row 64)
                    rec = nrm.tile([P, QW], F32, tag="rec")
                    nc.vector.reciprocal(out=rec[0:1, :], in_=acc[HD:HD + 1, :])
                    bc = nrm.tile([P, QW], F32, tag="bc")
                    nc.gpsimd.partition_broadcast(
                        bc[0:HD, :], rec[0:1, :], channels=HD)
                    nc.vector.tensor_mul(
                        out=att[jt][r0:r0 + HD, q0:q0 + QW],
                        in0=acc[0:HD, :],
                        in1=bc[0:HD, :],
                    )

        # ================= phase 3: output projection ==================
        with tc.tile_pool(name="ysb", bufs=3) as yp, \
             tc.tile_pool(name="ps3", bufs=4, space="PSUM") as ps3:
            for tt in range(TT):
                ysb = yp.tile([P, C], F32, tag="y")
                for nb in range(2):
                    ps = ps3.tile([P, QW], F32)
                    for ct in range(CT // 2):
                        nc.tensor.matmul(
                            ps[:, :],
                            att[ct][:, tt * P:(tt + 1) * P],
                            wpr[ct][:, nb * QW:(nb + 1) * QW],
                            start=(ct == 0), stop=(ct == CT // 2 - 1),
                        )
                    nc.vector.tensor_copy(
                        out=ysb[:, nb * QW:(nb + 1) * QW], in_=ps[:, :])
                nc.sync.dma_start(out=y[tt * P:(tt + 1) * P, :], in_=ysb[:, :])


def _get_compiled():
    global _COMPILED
    if _COMPILED is None:
        _COMPILED = _build_nc()
    return _COMPILED


def _make_in_maps(x, w_qkv, b_qkv, w_proj):
    in_maps = []
    for c in range(N_CORES):
        b, g = c // 2, c % 2
        s = slice(g * GC, (g + 1) * GC)
        in_maps.append({
            "xT": np.ascontiguousarray(x[b].T),
            "w_qk": np.ascontiguousarray(
                np.concatenate([w_qkv[:, s], w_qkv[:, C + g * GC:C + (g + 1) * GC]],
                               axis=1)),
            "w_v": np.ascontiguousarray(w_qkv[:, 2 * C + g * GC:2 * C + (g + 1) * GC]),
            "b_qk": np.ascontiguousarray(
                np.concatenate([b_qkv[s], b_qkv[C + g * GC:C + (g + 1) * GC]])),
            "b_v": np.ascontiguousarray(b_qkv[2 * C + g * GC:2 * C + (g + 1) * GC]),
            "w_pr": np.ascontiguousarray(w_proj[g * GC:(g + 1) * GC, :]),
        })
    return in_maps


def run(x, w_qkv, b_qkv, w_proj, b_proj, trace=False):
    nc = _get_compiled()
    in_maps = _make_in_maps(np.asarray(x, dtype=np.float32),
                            np.asarray(w_qkv, dtype=np.float32),
                            np.asarray(b_qkv, dtype=np.float32),
                            np.asarray(w_proj, dtype=np.float32))
    res = run_bass_kernel_spmd(nc, in_maps, list(range(N_CORES)), trace=trace)
    out = np.empty((B, T, C), dtype=np.float32)
    bp = np.asarray(b_proj, dtype=np.float32)
    for b in range(B):
        out[b] = res.results[2 * b]["y"] + res.results[2 * b + 1]["y"] + bp
    return out, res


def kernel(x, w_qkv, b_qkv, w_proj, b_proj):
    out, _ = run(x, w_qkv, b_qkv, w_proj, b_proj)
    return out
